# revision 1
# baseline (speedup 1.0000x reference)
# GAT (graph attention) layer on 8 Trainium2 NeuronCores.
#
# Strategy: target-sharded edges. Each core owns 1/8 of the target nodes and
# processes exactly the edges pointing into its range, so the segment-sum
# (softmax denominator + weighted feature aggregation) is core-local. The only
# collective is an AllReduce-max of one scalar (the global max attention
# score, needed to reproduce the reference's `exp(e - e.max())` + `+1e-16`
# epsilon numerics exactly).
#
# Per core:
#   Phase T: proj = x @ W and s_src per node -> gather tables A/B (bf16 rows
#     [proj(128) | s_hi(4) | s_lo(4) | pad], 512B, split at node 25000 so
#     dma_gather's int16 indices stay in range), plus a local s_trg table.
#   Phase E: edges host-sorted by target into windows of 128 target nodes,
#     each window's edges split-sorted by src half. Per window: two ucode
#     dma_gathers fetch [proj|s_src] rows; s_trg comes from a one-hot matmul
#     (host-provided selT) against the local s_trg window slice; then
#     e = leaky_relu(s_src + s_trg), ex = exp(e - 24), and two
#     PSUM-accumulated matmuls against a one-hot selection matrix (is_equal
#     vs an iota tile) give per-window weighted sums and denominators.
#     A running max of raw scores is kept on the side.
#   Collective: AllReduce(max) -> global M.
#   Phase F: out = elu(W/(D + 1e-16*exp(M-24)) + x + bias)  (identical to the
#     reference's shifted softmax + epsilon, since all sums carry exp(-24)).
import sys
from contextlib import ExitStack

import numpy as np

sys.path.insert(0, "/opt/trn_rl_repo")

import ml_dtypes  # noqa: E402

import concourse.bass as bass  # noqa: E402,F401
import concourse.mybir as mybir  # noqa: E402
import concourse.tile as tile  # noqa: E402
from concourse import bacc  # noqa: E402
from concourse.masks import make_identity  # noqa: E402

P = 128
NH, FOUT = 4, 32
NHF = NH * FOUT  # 128
FIN = 128
ROW = 2 * P  # gather-table row: 256 bf16 = 512B
LEAKY = 0.2
SHIFT = 24.0
F32 = mybir.dt.float32
BF16 = mybir.dt.bfloat16
I16 = mybir.dt.int16
AX = mybir.AxisListType
OP = mybir.AluOpType
ACT = mybir.ActivationFunctionType
BF = ml_dtypes.bfloat16


def _wrap16(flat):
    """[..., L] -> dma_gather layout [..., 16, L//16] replicated to 128 rows."""
    L = flat.shape[-1]
    w = flat.reshape(flat.shape[:-1] + (L // 16, 16))
    w = np.swapaxes(w, -1, -2)  # [..., 16, L//16]
    return np.tile(w, (1, 1, 8, 1)).reshape(flat.shape[:-1] + (P, L // 16))


def _prepare_edges(edge_index, n_nodes, n_cores):
    npc = n_nodes // n_cores
    nw = (npc + P - 1) // P
    half = n_nodes // 2
    src = np.ascontiguousarray(edge_index[0]).astype(np.int64)
    trg = np.ascontiguousarray(edge_index[1]).astype(np.int64)
    E = src.shape[0]
    wglob = (trg // npc) * nw + (trg % npc) // P
    isb = (src >= half).astype(np.int64)
    order = np.argsort(wglob * 2 + isb, kind="stable")
    src_s, trg_s, wg_s, isb_s = src[order], trg[order], wglob[order], isb[order]
    nwin = n_cores * nw
    cnt_a = np.bincount(wg_s * 2 + isb_s, minlength=2 * nwin)[0::2]
    cnt_b = np.bincount(wg_s * 2 + isb_s, minlength=2 * nwin)[1::2]
    t_a = max(1, int(np.ceil(cnt_a.max() / P)))
    t_b = max(1, int(np.ceil(cnt_b.max() / P)))
    t_eff = t_a + t_b
    # position within (window, half) group
    gkey = wg_s * 2 + isb_s
    gstart = np.concatenate([[0], np.cumsum(np.bincount(gkey, minlength=2 * nwin))])[:-1]
    jj = np.arange(E) - gstart[gkey]
    t_loc = jj // P
    p_idx = jj % P
    t_idx = np.where(isb_s == 1, t_a + t_loc, t_loc)
    c = wg_s // nw
    wloc = wg_s % nw
    rel = (trg_s % npc) - wloc * P  # 0..127

    idx_a = np.zeros((n_cores, nw, t_a * P), np.int16)
    idx_b = np.zeros((n_cores, nw, t_b * P), np.int16)
    ma = isb_s == 0
    idx_a[c[ma], wloc[ma], t_loc[ma] * P + p_idx[ma]] = src_s[ma].astype(np.int16)
    mb = isb_s == 1
    idx_b[c[mb], wloc[mb], t_loc[mb] * P + p_idx[mb]] = (src_s[mb] - half).astype(np.int16)

    rel_arr = np.full((n_cores, nw * P, t_eff), -1.0, np.float32)
    rel_arr[c, wloc * P + p_idx, t_idx] = rel
    selt = np.zeros((n_cores, nw * P, t_eff * P), BF)
    selt[c, wloc * P + rel, t_idx * P + p_idx] = 1.0

    ia = _wrap16(idx_a)  # [nc, nw, 128, t_a*8]
    ib = _wrap16(idx_b)
    gidx = np.concatenate([ia, ib], axis=-1).reshape(n_cores, nw * P, t_eff * 8)
    return t_a, t_b, gidx, rel_arr.astype(BF), selt


def build_bass(n_nodes, n_cores, t_a, t_b, debug_out=False):
    npc = n_nodes // n_cores
    nw = (npc + P - 1) // P
    nt = (n_nodes + P - 1) // P
    half = n_nodes // 2
    t_eff = t_a + t_b
    nc = bacc.Bacc("TRN2", target_bir_lowering=False, debug=False,
                   num_devices=n_cores)

    x = nc.dram_tensor("x", [n_nodes, FIN], F32, kind="ExternalInput")
    xloc = nc.dram_tensor("xloc", [npc, FIN], F32, kind="ExternalInput")
    w_in = nc.dram_tensor("W", [FIN, NHF], F32, kind="ExternalInput")
    amat = nc.dram_tensor("amat", [NHF, 2 * NH], F32, kind="ExternalInput")
    bias_in = nc.dram_tensor("bias", [1, NHF], F32, kind="ExternalInput")
    gidx = nc.dram_tensor("gidx", [nw * P, t_eff * 8], I16, kind="ExternalInput")
    trg_rel = nc.dram_tensor("trg_rel", [nw * P, t_eff], BF16,
                             kind="ExternalInput")
    selt_in = nc.dram_tensor("selt", [nw * P, t_eff * P], BF16,
                             kind="ExternalInput")
    out = nc.dram_tensor("out", [npc, NHF], F32, kind="ExternalOutput")

    dbgk = "ExternalOutput" if debug_out else "Internal"
    dbg = nc.dram_tensor("dbg", [8, P], F32, kind="ExternalOutput") if debug_out else None
    tab_a = nc.dram_tensor("tab_a", [half, ROW], BF16)
    tab_b = nc.dram_tensor("tab_b", [n_nodes - half, ROW], BF16)
    tab_c = nc.dram_tensor("tab_c", [nw * P, 2 * NH], BF16, kind=dbgk)
    acc_wt = nc.dram_tensor("acc_wt", [nw * P, P], F32, kind=dbgk)
    acc_d = nc.dram_tensor("acc_d", [nw * NH, P], F32, kind=dbgk)

    with tile.TileContext(nc) as tc, ExitStack() as ctx:
        const = ctx.enter_context(tc.tile_pool(name="const", bufs=1))
        sb = ctx.enter_context(tc.tile_pool(name="sb", bufs=3))
        sbg = ctx.enter_context(tc.tile_pool(name="sbg", bufs=2))
        dram = ctx.enter_context(tc.tile_pool(name="dram", bufs=1, space="DRAM"))

        ident = const.tile([P, P], F32)
        make_identity(nc, ident[:])
        c_i32 = const.tile([P, P], mybir.dt.int32)
        nc.gpsimd.iota(c_i32[:], pattern=[[1, P]], base=0, channel_multiplier=0)
        c_bf = const.tile([P, P], BF16)
        nc.vector.tensor_copy(c_bf[:], c_i32[:])

        sb_w = const.tile([FIN, NHF], F32)
        nc.sync.dma_start(sb_w[:], w_in[:])
        sb_a = const.tile([NHF, 2 * NH], F32)
        nc.sync.dma_start(sb_a[:], amat[:])
        sb_bias = const.tile([1, NHF], F32)
        nc.sync.dma_start(sb_bias[:], bias_in[:])

        with tc.tile_pool(name="ps0", bufs=1, space="PSUM") as ps0:
            ps_wt = ps0.tile([NHF, FIN], F32, tag="pst")
            nc.tensor.transpose(ps_wt[:], sb_w[:], ident[:])
            sb_wt = sb.tile([NHF, FIN], F32)
            nc.vector.tensor_copy(sb_wt[:], ps_wt[:])
            ps_wa = ps0.tile([FIN, 2 * NH], F32, tag="pst2")
            nc.tensor.matmul(ps_wa[:], lhsT=sb_wt[:], rhs=sb_a[:], start=True,
                             stop=True)
            wcat = const.tile([FIN, NHF + 2 * NH], F32)
            nc.vector.tensor_copy(wcat[:, 0:NHF], sb_w[:])
            nc.vector.tensor_copy(wcat[:, NHF:NHF + 2 * NH], ps_wa[:])
            ones_row = const.tile([1, P], F32)
            nc.gpsimd.memset(ones_row[:], 1.0)
            ps_b = ps0.tile([P, NHF], F32, tag="pst3")
            nc.tensor.matmul(ps_b[:], lhsT=ones_row[:], rhs=sb_bias[:],
                             start=True, stop=True)
            sb_b = const.tile([P, NHF], F32)
            nc.vector.tensor_copy(sb_b[:], ps_b[:])

        bias_zero = const.tile([P, 1], F32)
        nc.gpsimd.memset(bias_zero[:], 0.0)
        bias_mshift = const.tile([P, 1], F32)
        nc.gpsimd.memset(bias_mshift[:], -SHIFT)

        # --- phase T: gather tables (global) + local s_trg table ---
        with tc.tile_pool(name="psT", bufs=2, space="PSUM") as psT:
            for i in range(nt):
                r0 = i * P
                rows = min(P, n_nodes - r0)
                xt = sb.tile([P, FIN], F32, tag="xt")
                nc.sync.dma_start(xt[:rows], x[r0:r0 + rows, :])
                ps_xt = psT.tile([P, P], F32, tag="ps_xt")
                nc.tensor.transpose(ps_xt[:, :rows], xt[:rows, :],
                                    ident[:rows, :rows])
                x_tr = sb.tile([P, P], F32, tag="x_tr")
                nc.vector.tensor_copy(x_tr[:, :rows], ps_xt[:, :rows])
                ps_tab = psT.tile([P, NHF + NH], F32, tag="ps_tab")
                nc.tensor.matmul(ps_tab[:rows, :], lhsT=x_tr[:, :rows],
                                 rhs=wcat[:, 0:NHF + NH], start=True, stop=True)
                tabt = sb.tile([P, NHF + 2 * NH], BF16, tag="tabt")
                nc.vector.tensor_copy(tabt[:rows, 0:NHF], ps_tab[:rows, 0:NHF])
                nc.vector.tensor_copy(tabt[:rows, NHF:NHF + NH],
                                      ps_tab[:rows, NHF:NHF + NH])
                s_lo = sb.tile([P, NH], F32, tag="s_lo")
                nc.vector.tensor_tensor(s_lo[:rows], ps_tab[:rows, NHF:NHF + NH],
                                        tabt[:rows, NHF:NHF + NH], OP.subtract)
                nc.vector.tensor_copy(tabt[:rows, NHF + NH:NHF + 2 * NH],
                                      s_lo[:rows])
                # route rows to table A/B (split at `half`)
                if r0 + rows <= half:
                    nc.sync.dma_start(tab_a[r0:r0 + rows, 0:NHF + 2 * NH],
                                      tabt[:rows, :])
                elif r0 >= half:
                    nc.sync.dma_start(tab_b[r0 - half:r0 - half + rows,
                                            0:NHF + 2 * NH], tabt[:rows, :])
                else:
                    k = half - r0
                    nc.sync.dma_start(tab_a[r0:half, 0:NHF + 2 * NH], tabt[:k, :])
                    nc.sync.dma_start(tab_b[0:r0 + rows - half, 0:NHF + 2 * NH],
                                      tabt[k:rows, :])
            # local s_trg table from xloc
            for i in range(nw):
                r0 = i * P
                rows = min(P, npc - r0)
                xt = sb.tile([P, FIN], F32, tag="xt")
                nc.sync.dma_start(xt[:rows], xloc[r0:r0 + rows, :])
                ps_xt = psT.tile([P, P], F32, tag="ps_xt")
                nc.tensor.transpose(ps_xt[:, :rows], xt[:rows, :],
                                    ident[:rows, :rows])
                x_tr = sb.tile([P, P], F32, tag="x_tr")
                nc.vector.tensor_copy(x_tr[:, :rows], ps_xt[:, :rows])
                ps_c = psT.tile([P, NH], F32, tag="ps_c")
                nc.tensor.matmul(ps_c[:rows, :], lhsT=x_tr[:, :rows],
                                 rhs=wcat[:, NHF + NH:NHF + 2 * NH], start=True,
                                 stop=True)
                tabc = sb.tile([P, 2 * NH], BF16, tag="tabc")
                if rows < P:
                    nc.gpsimd.memset(tabc[:], 0.0)
                nc.vector.tensor_copy(tabc[:rows, 0:NH], ps_c[:rows, :])
                c_lo = sb.tile([P, NH], F32, tag="c_lo")
                nc.vector.tensor_tensor(c_lo[:rows], ps_c[:rows, :],
                                        tabc[:rows, 0:NH], OP.subtract)
                nc.vector.tensor_copy(tabc[:rows, NH:2 * NH], c_lo[:rows])
                nc.sync.dma_start(tab_c[r0:r0 + P, :], tabc[:])

        # --- phase E ---
        zmax = const.tile([P, t_eff * NH], F32)
        nc.gpsimd.memset(zmax[:], -1e30)
        psE = ctx.enter_context(tc.tile_pool(name="psE", bufs=2, space="PSUM"))

        for w in range(nw):
            er0 = w * P
            idx_t = sb.tile([P, t_eff * 8], I16, tag="idx_t")
            nc.sync.dma_start(idx_t[:], gidx[er0:er0 + P, :])
            rel = sb.tile([P, t_eff], BF16, tag="rel")
            nc.sync.dma_start(rel[:], trg_rel[er0:er0 + P, :])
            selt = sbg.tile([P, t_eff * P], BF16, tag="selt")
            nc.sync.dma_start(selt[:], selt_in[er0:er0 + P, :])
            stw = sb.tile([P, 2 * NH], BF16, tag="stw")
            nc.sync.dma_start(stw[:], tab_c[er0:er0 + P, :])

            gath = sbg.tile([P, t_eff * ROW], BF16, tag="gath")
            g3 = gath[:].rearrange("p (t c) -> p t c", c=ROW)
            nc.gpsimd.dma_gather(
                out_ap=g3[:, 0:t_a, :], in_ap=tab_a[:],
                idxs_ap=idx_t[:, 0:t_a * 8], num_idxs=t_a * P,
                num_idxs_reg=t_a * P, elem_size=ROW, single_packet=False)
            nc.gpsimd.dma_gather(
                out_ap=g3[:, t_a:t_eff, :], in_ap=tab_b[:],
                idxs_ap=idx_t[:, t_a * 8:], num_idxs=t_b * P,
                num_idxs_reg=t_b * P, elem_size=ROW, single_packet=False)

            # s_trg per edge via one-hot matmul (batched into one PSUM bank)
            ps_st = psE.tile([P, t_eff * 2 * NH], F32, tag="ps_st")
            for t in range(t_eff):
                nc.tensor.matmul(ps_st[:, t * 2 * NH:(t + 1) * 2 * NH],
                                 lhsT=selt[:, t * P:(t + 1) * P], rhs=stw[:],
                                 start=True, stop=True)
            st3 = ps_st[:].rearrange("p (t h) -> p t h", h=2 * NH)

            z = sb.tile([P, t_eff * NH], F32, tag="z")
            z3 = z[:].rearrange("p (t h) -> p t h", h=NH)
            nc.vector.tensor_tensor(z3, g3[:, :, NHF:NHF + NH],
                                    g3[:, :, NHF + NH:NHF + 2 * NH], OP.add)
            nc.vector.tensor_tensor(z3, z3, st3[:, :, 0:NH], OP.add)
            nc.vector.tensor_tensor(z3, z3, st3[:, :, NH:2 * NH], OP.add)
            nc.vector.tensor_tensor(zmax[:], zmax[:], z[:], OP.max)
            zl = sb.tile([P, t_eff * NH], F32, tag="zl")
            nc.vector.tensor_scalar_mul(zl[:], z[:], LEAKY)
            nc.vector.tensor_tensor(zl[:], zl[:], z[:], OP.max)
            ex = sb.tile([P, t_eff * NH], BF16, tag="ex")
            nc.scalar.activation(ex[:], zl[:], ACT.Exp, bias=bias_mshift[:])

            sel = sbg.tile([P, t_eff * P], BF16, tag="sel")
            nc.vector.tensor_tensor(
                sel[:].rearrange("p (t q) -> p t q", q=P),
                rel[:, :, None].to_broadcast([P, t_eff, P]),
                c_bf[:, None, :].to_broadcast([P, t_eff, P]),
                OP.is_equal)
            wgt = sbg.tile([P, t_eff * NHF], BF16, tag="wgt")
            nc.vector.tensor_tensor(
                wgt[:].rearrange("p (t h f) -> p t h f", h=NH, f=FOUT),
                g3[:, :, 0:NHF].rearrange("p t (h f) -> p t h f", f=FOUT),
                ex[:].rearrange("p (t h) -> p t h", h=NH)[:, :, :, None]
                .to_broadcast([P, t_eff, NH, FOUT]),
                OP.mult)

            ps_w = psE.tile([P, P], F32, tag="ps_w")
            ps_d = psE.tile([NH, P], F32, tag="ps_d")
            for t in range(t_eff):
                nc.tensor.matmul(ps_w[:], lhsT=wgt[:, t * NHF:(t + 1) * NHF],
                                 rhs=sel[:, t * P:(t + 1) * P],
                                 start=(t == 0), stop=(t == t_eff - 1))
                nc.tensor.matmul(ps_d[:], lhsT=ex[:, t * NH:(t + 1) * NH],
                                 rhs=sel[:, t * P:(t + 1) * P],
                                 start=(t == 0), stop=(t == t_eff - 1))
            wt_sb = sb.tile([P, P], F32, tag="wt_sb")
            nc.vector.tensor_copy(wt_sb[:], ps_w[:])
            d_sb = sb.tile([NH, P], F32, tag="d_sb")
            nc.vector.tensor_copy(d_sb[:], ps_d[:])
            nc.sync.dma_start(acc_wt[w * P:(w + 1) * P, :], wt_sb[:])
            nc.sync.dma_start(acc_d[w * NH:(w + 1) * NH, :], d_sb[:])

        # --- global max + epsilon scalar ---
        zm1 = sb.tile([P, 1], F32, tag="zm1")
        nc.vector.tensor_reduce(zm1[:], zmax[:], axis=AX.X, op=OP.max)
        zm0 = sb.tile([1, 1], F32, tag="zm0")
        nc.gpsimd.tensor_reduce(zm0[:], zm1[:], axis=AX.C, op=OP.max)
        cc_in = dram.tile([1, 1], F32)
        cc_out = dram.tile([1, 1], F32)
        nc.sync.dma_start(cc_in[:], zm0[:])
        nc.gpsimd.collective_compute(
            "AllReduce", OP.max, replica_groups=[list(range(n_cores))],
            ins=[cc_in.opt()], outs=[cc_out.opt()])
        zg = sb.tile([1, 1], F32, tag="zg")
        nc.sync.dma_start(zg[:], cc_out[:])
        eg = sb.tile([1, 1], F32, tag="eg")
        nc.vector.tensor_scalar_mul(eg[:], zg[:], LEAKY)
        nc.vector.tensor_tensor(eg[:], eg[:], zg[:], OP.max)
        ce = sb.tile([1, 1], F32, tag="ce")
        nc.scalar.activation(ce[:], eg[:], ACT.Exp, bias=bias_mshift[:1])
        nc.vector.tensor_scalar_mul(ce[:], ce[:], 1e-16)
        ceps = const.tile([P, 1], F32)
        nc.gpsimd.partition_broadcast(ceps[:], ce[:])

        if dbg is not None:
            dbg_t = sb.tile([1, P], F32, tag="dbg_t")
            nc.gpsimd.memset(dbg_t[:], 0.0)
            nc.vector.tensor_copy(dbg_t[:, 0:1], zm0[:])
            nc.vector.tensor_copy(dbg_t[:, 1:2], zg[:])
            nc.vector.tensor_copy(dbg_t[:, 2:3], eg[:])
            nc.vector.tensor_copy(dbg_t[:, 3:4], ce[:])
            nc.sync.dma_start(dbg[0:1, :], dbg_t[:])

        # --- phase F ---
        for w in range(nw):
            rows = min(P, npc - w * P)
            wt_l = sb.tile([P, P], F32, tag="wt_l")
            nc.sync.dma_start(wt_l[:], acc_wt[w * P:(w + 1) * P, :])
            dt_l = sb.tile([NH, P], F32, tag="dt_l")
            nc.sync.dma_start(dt_l[:], acc_d[w * NH:(w + 1) * NH, :])
            ps_w2 = psE.tile([P, P], F32, tag="ps_w")
            nc.tensor.transpose(ps_w2[:], wt_l[:], ident[:])
            ps_d2 = psE.tile([P, NH], F32, tag="ps_d")
            nc.tensor.transpose(ps_d2[:], dt_l[:], ident[:NH, :NH])
            den = sb.tile([P, NH], F32, tag="den")
            nc.vector.tensor_tensor(den[:], ps_d2[:],
                                    ceps[:, :1].to_broadcast([P, NH]), OP.add)
            rec = sb.tile([P, NH], F32, tag="rec")
            nc.vector.reciprocal(rec[:], den[:])
            o1 = sb.tile([P, NHF], F32, tag="o1")
            nc.vector.tensor_tensor(
                o1[:].rearrange("p (h f) -> p h f", f=FOUT),
                ps_w2[:].rearrange("p (h f) -> p h f", f=FOUT),
                rec[:, :, None].to_broadcast([P, NH, FOUT]),
                OP.mult)
            xw = sb.tile([P, NHF], F32, tag="xw")
            nc.sync.dma_start(xw[:rows], xloc[w * P:w * P + rows, :])
            nc.vector.tensor_tensor(o1[:rows], o1[:rows], xw[:rows], OP.add)
            nc.vector.tensor_tensor(o1[:rows], o1[:rows], sb_b[:rows], OP.add)
            nmin = sb.tile([P, NHF], F32, tag="nmin")
            nc.vector.tensor_scalar(nmin[:rows], o1[:rows], 0.0, None, OP.min)
            en = sb.tile([P, NHF], F32, tag="en")
            nc.scalar.activation(en[:rows], nmin[:rows], ACT.Exp,
                                 bias=bias_zero[:rows])
            pos = sb.tile([P, NHF], F32, tag="pos")
            nc.vector.tensor_scalar(pos[:rows], o1[:rows], 0.0, None, OP.max)
            nc.vector.tensor_tensor(en[:rows], en[:rows], pos[:rows], OP.add)
            nc.vector.tensor_scalar(en[:rows], en[:rows], -1.0, None, OP.add)
            nc.sync.dma_start(out[w * P:w * P + rows, :], en[:rows])

    nc.compile()
    return nc


def _make_inputs(x, edge_index, w_mat, a_src, a_trg, bias, n_cores):
    n_nodes = x.shape[0]
    npc = n_nodes // n_cores
    t_a, t_b, gidx, rel_arr, selt = _prepare_edges(edge_index, n_nodes, n_cores)
    amat = np.zeros((NHF, 2 * NH), np.float32)
    for h in range(NH):
        amat[h * FOUT:(h + 1) * FOUT, h] = a_src[h]
        amat[h * FOUT:(h + 1) * FOUT, NH + h] = a_trg[h]
    x = np.ascontiguousarray(x, dtype=np.float32)
    in_maps = []
    for c in range(n_cores):
        in_maps.append({
            "x": x,
            "xloc": np.ascontiguousarray(x[c * npc:(c + 1) * npc]),
            "W": np.ascontiguousarray(w_mat, dtype=np.float32),
            "amat": amat,
            "bias": np.ascontiguousarray(bias, dtype=np.float32).reshape(1, NHF),
            "gidx": np.ascontiguousarray(gidx[c]),
            "trg_rel": np.ascontiguousarray(rel_arr[c]),
            "selt": np.ascontiguousarray(selt[c]),
        })
    return t_a, t_b, in_maps


def kernel(x, edge_index, W, a_src, a_trg, bias, _trace=False):
    from concourse.bass_utils import run_bass_kernel_spmd

    n_cores = 8
    x = np.asarray(x)
    n_nodes = x.shape[0]
    t_a, t_b, in_maps = _make_inputs(np.asarray(x), np.asarray(edge_index),
                                     np.asarray(W), np.asarray(a_src),
                                     np.asarray(a_trg), np.asarray(bias),
                                     n_cores)
    nc = build_bass(n_nodes, n_cores, t_a, t_b)
    res = run_bass_kernel_spmd(nc, in_maps, core_ids=list(range(n_cores)),
                               trace=_trace)
    out = np.concatenate([res.results[c]["out"] for c in range(n_cores)], axis=0)
    if _trace:
        kernel.last_results = res
    return out.astype(np.float32)

